# revision 1
# baseline (speedup 1.0000x reference)
"""Bone_Direction_GCN fused kernel for 8 Trainium2 NeuronCores.

Data-parallel over the batch dim: each core processes 2048 of 16384 batches.
All graph mixing (GCN conv + dense-adj einsum) is expressed as block-diagonal
matmuls over groups of 7 batches (7*17 = 119 rows <= 128 partitions), fully
fused with the channel matmuls on the PE array in bf16 (fp32 residual).
"""

import sys

sys.path.insert(0, "/opt/trn_rl_repo")

import numpy as np
import ml_dtypes

B, J, E = 16384, 17, 32
CIN, COUT = 128, 128
MID = COUT // 2
PROP = 0.5
SLOPE = 0.01

N_CORES = 8
BC = B // N_CORES          # batches per core (2048)
ROWS = BC * J              # rows per core (34816)
G = 7                      # batches per sub-tile
R = G * J                  # rows per sub-tile (119)
S = 4                      # sub-tiles per macro-tile
RM = S * R                 # rows per macro-tile (476)
NM = 73                    # macro tiles per core (73*476 = 34748)
GT = BC - NM * S * G       # tail batches (4)
RT = GT * J                # tail rows (68)

assert NM * RM + RT == ROWS

_CACHE = {}


def _gcn_matrix(edge_index: np.ndarray, edge_weight: np.ndarray) -> np.ndarray:
    """Dense normalized GCN operator M with out[i] = sum_j M[i, j] * x[j]."""
    row = edge_index[0].astype(np.int64)
    col = edge_index[1].astype(np.int64)
    loop = np.arange(J, dtype=np.int64)
    row_f = np.concatenate([row, loop])
    col_f = np.concatenate([col, loop])
    w_f = np.concatenate([edge_weight.astype(np.float32), np.ones(J, np.float32)])
    deg = np.zeros(J, np.float32)
    np.add.at(deg, col_f, w_f)
    safe = np.where(deg > 0, deg, 1.0).astype(np.float32)
    dis = np.where(deg > 0, 1.0 / np.sqrt(safe), 0.0).astype(np.float32)
    norm = dis[row_f] * w_f * dis[col_f]
    M = np.zeros((J, J), np.float32)
    np.add.at(M, (col_f, row_f), norm)
    return M


def _block_diag(block: np.ndarray, n: int) -> np.ndarray:
    j = block.shape[0]
    out = np.zeros((n * j, n * j), block.dtype)
    for g in range(n):
        out[g * j:(g + 1) * j, g * j:(g + 1) * j] = block
    return out


def _mix_consts(M: np.ndarray, adj: np.ndarray, g: int):
    """mixI [g*17, 2*g*17] = [blockdiag(M.T) | I]; mix2e [g*17+2, g*17]."""
    r = g * J
    mix1 = _block_diag(M.T, g)
    mixI = np.concatenate([mix1, np.eye(r, dtype=np.float32)], axis=1)
    mix2 = _block_diag(PROP * adj, g)
    ones_row = np.ones((1, r), np.float32)
    s_row = np.tile(PROP * adj.sum(axis=0), g)[None, :]
    mix2e = np.concatenate([mix2, ones_row, s_row], axis=0)
    return mixI, mix2e


def _build_bass(leaky_mode: str = "lrelu", **_ignored):
    import concourse.bacc as bacc
    import concourse.mybir as mybir
    import concourse.tile as tile
    from contextlib import ExitStack

    f32 = mybir.dt.float32
    bf16 = mybir.dt.bfloat16

    nc = bacc.Bacc("TRN2", target_bir_lowering=False, debug=False)

    x_d = nc.dram_tensor("x", [ROWS, CIN], f32, kind="ExternalInput").ap()
    mixI_d = nc.dram_tensor("mixI", [R, 2 * R], bf16, kind="ExternalInput").ap()
    mix2e_d = nc.dram_tensor("mix2e", [R + 2, R], bf16, kind="ExternalInput").ap()
    mixIt_d = nc.dram_tensor("mixIt", [RT, 2 * RT], bf16, kind="ExternalInput").ap()
    mix2et_d = nc.dram_tensor("mix2et", [RT + 2, RT], bf16, kind="ExternalInput").ap()
    w1_d = nc.dram_tensor("w1", [CIN, COUT], bf16, kind="ExternalInput").ap()
    w2t_d = nc.dram_tensor("w2t", [CIN, MID], bf16, kind="ExternalInput").ap()
    w4t_d = nc.dram_tensor("w4t", [MID, COUT], bf16, kind="ExternalInput").ap()
    b2_d = nc.dram_tensor("b2", [MID, 1], f32, kind="ExternalInput").ap()
    ab2_d = nc.dram_tensor("ab2", [MID, 1], f32, kind="ExternalInput").ap()
    b1b4_d = nc.dram_tensor("b1b4", [2, S * COUT], bf16, kind="ExternalInput").ap()
    o_d = nc.dram_tensor("out", [ROWS, CIN], f32, kind="ExternalOutput").ap()

    with ExitStack() as ctx:
        tc = ctx.enter_context(tile.TileContext(nc))

        const = ctx.enter_context(tc.tile_pool(name="const", bufs=1))
        mixI_sb = const.tile_from(mixI_d)
        mix2e_sb = const.tile_from(mix2e_d)
        mixIt_sb = const.tile_from(mixIt_d)
        mix2et_sb = const.tile_from(mix2et_d)
        w1_sb = const.tile_from(w1_d)
        w2t_sb = const.tile_from(w2t_d)
        w4t_sb = const.tile_from(w4t_d)
        b2_sb = const.tile_from(b2_d)
        ab2_sb = const.tile_from(ab2_d)

        def leaky(hbf, psH):
            if leaky_mode == "lrelu":
                nc.scalar.activation(
                    hbf[:], psH[:],
                    func=mybir.ActivationFunctionType.Lrelu,
                    bias=b2_sb[:], scale=1.0, alpha=SLOPE,
                )
            else:
                a = h_pool.tile(list(psH.shape), bf16, tag="lk_a")
                nc.scalar.activation(
                    a[:], psH[:],
                    func=mybir.ActivationFunctionType.Identity,
                    bias=ab2_sb[:], scale=SLOPE,
                )
                nc.vector.scalar_tensor_tensor(
                    hbf[:], psH[:], b2_sb[:], a[:],
                    op0=mybir.AluOpType.add, op1=mybir.AluOpType.max,
                )

        y2e_pool = ctx.enter_context(tc.tile_pool(name="y2e", bufs=2))
        y2e_tiles = []
        for i in range(2):
            t = y2e_pool.tile([R + 2, S * COUT], bf16, tag=f"y2e{i}")
            nc.sync.dma_start(out=t[R:R + 2, :], in_=b1b4_d)
            y2e_tiles.append(t)
        y2et_pool = ctx.enter_context(tc.tile_pool(name="y2et", bufs=1))
        y2et = y2et_pool.tile([RT + 2, COUT], bf16)
        nc.sync.dma_start(out=y2et[RT:RT + 2, :], in_=b1b4_d[:, 0:COUT])

        xin_pool = ctx.enter_context(tc.tile_pool(name="xin", bufs=3))
        xbf_pool = ctx.enter_context(tc.tile_pool(name="xbf", bufs=3))
        xm_pool = ctx.enter_context(tc.tile_pool(name="xm", bufs=2))
        xt_pool = ctx.enter_context(tc.tile_pool(name="xt", bufs=2))
        h_pool = ctx.enter_context(tc.tile_pool(name="h", bufs=2))
        out_pool = ctx.enter_context(tc.tile_pool(name="osb", bufs=3))

        psT_pool = ctx.enter_context(tc.tile_pool(name="psT", bufs=1, space="PSUM"))
        psH_pool = ctx.enter_context(tc.tile_pool(name="psH", bufs=2, space="PSUM"))
        psY2_pool = ctx.enter_context(tc.tile_pool(name="psY2", bufs=1, space="PSUM"))
        psO_pool = ctx.enter_context(tc.tile_pool(name="psO", bufs=1, space="PSUM"))

        for m in range(NM):
            r0 = m * RM
            xin = xin_pool.tile([R, S * CIN], f32)
            nc.sync.dma_start(
                out=xin[:].rearrange("p (s c) -> p s c", c=CIN),
                in_=x_d[r0:r0 + RM, :].rearrange("(s p) c -> p s c", p=R),
            )
            xbf = xbf_pool.tile([R, S * CIN], bf16)
            nc.gpsimd.tensor_copy(xbf[:], xin[:])

            xm = xm_pool.tile([CIN, S * R], bf16)
            xt = xt_pool.tile([CIN, S * R], bf16)
            psT = psT_pool.tile([CIN, S * 512], f32)
            for s in range(S):
                nc.tensor.matmul(
                    psT[:, s * 512:s * 512 + 2 * R],
                    lhsT=xbf[:, s * CIN:(s + 1) * CIN],
                    rhs=mixI_sb[:],
                    start=True, stop=True,
                )
                nc.vector.tensor_copy(
                    xm[:, s * R:(s + 1) * R], psT[:, s * 512:s * 512 + R])
                nc.scalar.copy(
                    xt[:, s * R:(s + 1) * R], psT[:, s * 512 + R:s * 512 + 2 * R])

            psH = psH_pool.tile([MID, RM], f32)
            for s in range(S):
                nc.tensor.matmul(
                    psH[:, s * R:(s + 1) * R],
                    lhsT=w2t_sb[:], rhs=xt[:, s * R:(s + 1) * R],
                    start=True, stop=True,
                )
            hbf = h_pool.tile([MID, RM], bf16)
            leaky(hbf, psH)
            psY2 = psY2_pool.tile([R, S * COUT], f32)
            for s in range(S):
                nc.tensor.matmul(
                    psY2[:, s * COUT:(s + 1) * COUT],
                    lhsT=hbf[:, s * R:(s + 1) * R], rhs=w4t_sb[:],
                    start=True, stop=True,
                )
            y2e = y2e_tiles[m % 2]
            nc.scalar.copy(y2e[0:R, :], psY2[:])

            psO = psO_pool.tile([R, S * COUT], f32)
            for s in range(S):
                nc.tensor.matmul(
                    psO[:, s * COUT:(s + 1) * COUT],
                    lhsT=xm[:, s * R:(s + 1) * R], rhs=w1_sb[:],
                    start=True, stop=False, skip_group_check=True,
                )
                nc.tensor.matmul(
                    psO[:, s * COUT:(s + 1) * COUT],
                    lhsT=mix2e_sb[:], rhs=y2e[:, s * COUT:(s + 1) * COUT],
                    start=False, stop=True, skip_group_check=True,
                )
            out_sb = out_pool.tile([R, S * CIN], f32)
            nc.vector.tensor_add(out_sb[:], psO[:], xin[:])
            nc.sync.dma_start(
                out=o_d[r0:r0 + RM, :].rearrange("(s p) c -> p s c", p=R),
                in_=out_sb[:].rearrange("p (s c) -> p s c", c=CIN),
            )

        r0 = NM * RM
        xin = xin_pool.tile([RT, CIN], f32, tag="xin")
        nc.sync.dma_start(out=xin[:], in_=x_d[r0:r0 + RT, :])
        xbf = xbf_pool.tile([RT, CIN], bf16, tag="xbf")
        nc.gpsimd.tensor_copy(xbf[:], xin[:])
        psT = psT_pool.tile([CIN, 2 * RT], f32, tag="psT")
        nc.tensor.matmul(psT[:], lhsT=xbf[:], rhs=mixIt_sb[:], start=True, stop=True)
        xm = xm_pool.tile([CIN, RT], bf16, tag="xm")
        nc.vector.tensor_copy(xm[:], psT[:, 0:RT])
        xt = xt_pool.tile([CIN, RT], bf16, tag="xt")
        nc.scalar.copy(xt[:], psT[:, RT:2 * RT])
        psH = psH_pool.tile([MID, RT], f32, tag="psH")
        nc.tensor.matmul(psH[:], lhsT=w2t_sb[:], rhs=xt[:], start=True, stop=True)
        hbf = h_pool.tile([MID, RT], bf16, tag="hbf")
        leaky(hbf, psH)
        psY2 = psY2_pool.tile([RT, COUT], f32, tag="psY2")
        nc.tensor.matmul(psY2[:], lhsT=hbf[:], rhs=w4t_sb[:], start=True, stop=True)
        nc.scalar.copy(y2et[0:RT, :], psY2[:])
        psO = psO_pool.tile([RT, COUT], f32, tag="psO")
        nc.tensor.matmul(psO[:], lhsT=xm[:], rhs=w1_sb[:],
                         start=True, stop=False, skip_group_check=True)
        nc.tensor.matmul(psO[:], lhsT=mix2et_sb[:], rhs=y2et[:],
                         start=False, stop=True, skip_group_check=True)
        out_sb = out_pool.tile([RT, CIN], f32, tag="out_sb")
        nc.vector.tensor_add(out_sb[:], psO[:], xin[:])
        nc.sync.dma_start(out=o_d[r0:r0 + RT, :], in_=out_sb[:])

    nc.compile()
    return nc


def _host_consts(inputs):
    bf = ml_dtypes.bfloat16
    M = _gcn_matrix(np.asarray(inputs["edge_index"]), np.asarray(inputs["edge_weight"]))
    adj = np.asarray(inputs["adj"], np.float32)
    mixI, mix2e = _mix_consts(M, adj, G)
    mixIt, mix2et = _mix_consts(M, adj, GT)
    W1 = np.asarray(inputs["W1"], np.float32)
    W2 = np.asarray(inputs["W2"], np.float32)
    W4 = np.asarray(inputs["W4"], np.float32)
    b1 = np.asarray(inputs["b1"], np.float32)
    b2 = np.asarray(inputs["b2"], np.float32)
    b4 = np.asarray(inputs["b4"], np.float32)
    b1b4 = np.stack([np.tile(b1, S), np.tile(b4, S)])
    return {
        "mixI": mixI.astype(bf),
        "mix2e": mix2e.astype(bf),
        "mixIt": mixIt.astype(bf),
        "mix2et": mix2et.astype(bf),
        "w1": np.ascontiguousarray(W1).astype(bf),
        "w2t": np.ascontiguousarray(W2.T).astype(bf),
        "w4t": np.ascontiguousarray(W4.T).astype(bf),
        "b2": np.ascontiguousarray(b2[:, None]),
        "ab2": np.ascontiguousarray(SLOPE * b2[:, None]),
        "b1b4": b1b4.astype(bf),
    }


def kernel(**inputs) -> np.ndarray:
    from concourse.bass_utils import run_bass_kernel_spmd

    if "nc" not in _CACHE:
        _CACHE["nc"] = _build_bass()
    nc = _CACHE["nc"]

    consts = _host_consts(inputs)
    vector = np.ascontiguousarray(np.asarray(inputs["vector"], np.float32))
    in_maps = []
    for c in range(N_CORES):
        m = dict(consts)
        m["x"] = np.ascontiguousarray(
            vector[c * BC:(c + 1) * BC].reshape(ROWS, CIN)
        )
        in_maps.append(m)

    res = run_bass_kernel_spmd(nc, in_maps, core_ids=list(range(N_CORES)))
    outs = [res.results[c]["out"].reshape(BC, J, CIN) for c in range(N_CORES)]
    return np.concatenate(outs, axis=0)



# revision 6
# speedup vs baseline: 2.9651x; 2.9651x over previous
"""Bone_Direction_GCN fused kernel for 8 Trainium2 NeuronCores.

Data-parallel over the batch dim: each core processes 2048 of 16384 batches.
x is shipped to the device as bf16 channel-major [CIN, rows] so both the input
and output DMAs move large contiguous per-partition chunks (~7.6KB packets).
Graph mixing (GCN conv + dense-adj einsum) is expressed as block-diagonal
"mixing transpose" matmuls over groups of 7 batches (7*17 = 119 rows), which
also return the result to channel-major layout for the residual add and the
bf16 channel-major output store.
"""

import sys

sys.path.insert(0, "/opt/trn_rl_repo")

import numpy as np
import ml_dtypes

B, J, E = 16384, 17, 32
CIN, COUT = 128, 128
MID = COUT // 2
PROP = 0.5
SLOPE = 0.01

N_CORES = 8
BC = B // N_CORES          # batches per core (2048)
ROWS = BC * J              # rows per core (34816)
G = 7                      # batches per group
R = G * J                  # rows per group (119)
NG = BC // G               # full groups per core (292)
GT = BC - NG * G           # tail batches (4)
RT = GT * J                # tail rows (68)
SGS = 4                    # groups per supergroup
RSG = SGS * R              # cols per supergroup (476)
NSG = NG // SGS            # supergroups per core (73)
TILE_SG = 8                # supergroups per DMA tile
NT_FULL = (NSG - 1) // TILE_SG  # 9 full tiles; last tile = 1 SG + tail
COLS_FULL = TILE_SG * RSG  # 3808
COLS_LAST = RSG + RT       # 544

assert NT_FULL * COLS_FULL + COLS_LAST == ROWS

_CACHE = {}


def _gcn_matrix(edge_index: np.ndarray, edge_weight: np.ndarray) -> np.ndarray:
    """Dense normalized GCN operator M with out[i] = sum_j M[i, j] * x[j]."""
    row = edge_index[0].astype(np.int64)
    col = edge_index[1].astype(np.int64)
    loop = np.arange(J, dtype=np.int64)
    row_f = np.concatenate([row, loop])
    col_f = np.concatenate([col, loop])
    w_f = np.concatenate([edge_weight.astype(np.float32), np.ones(J, np.float32)])
    deg = np.zeros(J, np.float32)
    np.add.at(deg, col_f, w_f)
    safe = np.where(deg > 0, deg, 1.0).astype(np.float32)
    dis = np.where(deg > 0, 1.0 / np.sqrt(safe), 0.0).astype(np.float32)
    norm = dis[row_f] * w_f * dis[col_f]
    M = np.zeros((J, J), np.float32)
    np.add.at(M, (col_f, row_f), norm)
    return M


def _block_diag(block: np.ndarray, n: int) -> np.ndarray:
    j = block.shape[0]
    out = np.zeros((n * j, n * j), block.dtype)
    for g in range(n):
        out[g * j:(g + 1) * j, g * j:(g + 1) * j] = block
    return out


def _mix_consts(M: np.ndarray, adj: np.ndarray, g: int):
    """bdM [g*17, g*17] = blockdiag(M.T); m2e [g*17+2, g*17] = mix2 + bias rows.

    psF[c, r'] += sum_r y1u[r, c] * bdM[r, r']   (GCN mix, row r -> row r')
    psF[c, r'] += sum_k y2e[k, c] * m2e[k, r']   (adj mix + b1/b4 bias rows)
    """
    r = g * J
    bdM = _block_diag(M.T, g)
    mix2 = _block_diag(PROP * adj, g)
    ones_row = np.ones((1, r), np.float32)
    s_row = np.tile(PROP * adj.sum(axis=0), g)[None, :]
    m2e = np.concatenate([mix2, ones_row, s_row], axis=0)
    return bdM, m2e


def _build_bass(leaky_mode: str = "lrelu", **_ignored):
    import concourse.bacc as bacc
    import concourse.mybir as mybir
    import concourse.tile as tile
    from contextlib import ExitStack

    f32 = mybir.dt.float32
    bf16 = mybir.dt.bfloat16

    nc = bacc.Bacc("TRN2", target_bir_lowering=False, debug=False)

    x_d = nc.dram_tensor("x", [CIN, ROWS], bf16, kind="ExternalInput").ap()
    w1_d = nc.dram_tensor("w1", [CIN, COUT], bf16, kind="ExternalInput").ap()
    w2t_d = nc.dram_tensor("w2t", [CIN, MID], bf16, kind="ExternalInput").ap()
    w4t_d = nc.dram_tensor("w4t", [MID, COUT], bf16, kind="ExternalInput").ap()
    b2_d = nc.dram_tensor("b2", [MID, 1], f32, kind="ExternalInput").ap()
    ab2_d = nc.dram_tensor("ab2", [MID, 1], f32, kind="ExternalInput").ap()
    bdM_d = nc.dram_tensor("bdM", [R, R], bf16, kind="ExternalInput").ap()
    m2e_d = nc.dram_tensor("m2e", [R + 2, R], bf16, kind="ExternalInput").ap()
    bdM4_d = nc.dram_tensor("bdM4", [RT, RT], bf16, kind="ExternalInput").ap()
    m2e4_d = nc.dram_tensor("m2e4", [RT + 2, RT], bf16, kind="ExternalInput").ap()
    b1b4_d = nc.dram_tensor("b1b4", [2, 4 * COUT], bf16, kind="ExternalInput").ap()
    o_d = nc.dram_tensor("out", [CIN, ROWS], bf16, kind="ExternalOutput").ap()

    with ExitStack() as ctx:
        tc = ctx.enter_context(tile.TileContext(nc))

        const = ctx.enter_context(tc.tile_pool(name="const", bufs=1))
        w1_sb = const.tile_from(w1_d)
        w2t_sb = const.tile_from(w2t_d)
        w4t_sb = const.tile_from(w4t_d)
        b2_sb = const.tile_from(b2_d)
        ab2_sb = const.tile_from(ab2_d)
        bdM_sb = const.tile_from(bdM_d)
        m2e_sb = const.tile_from(m2e_d)
        bdM4_sb = const.tile_from(bdM4_d)
        m2e4_sb = const.tile_from(m2e4_d)

        def leaky(hbf, psH):
            if leaky_mode == "lrelu":
                nc.scalar.activation(
                    hbf[:], psH[:],
                    func=mybir.ActivationFunctionType.Lrelu,
                    bias=b2_sb[:], scale=1.0, alpha=SLOPE,
                )
            else:
                a = lk_pool.tile(list(psH.shape), bf16, tag="lk_a")
                nc.scalar.activation(
                    a[:], psH[:],
                    func=mybir.ActivationFunctionType.Identity,
                    bias=ab2_sb[:], scale=SLOPE,
                )
                nc.vector.scalar_tensor_tensor(
                    hbf[:], psH[:], b2_sb[:], a[:],
                    op0=mybir.AluOpType.add, op1=mybir.AluOpType.max,
                )

        # y2e tiles: rows 0:R hold y2 (d before bias/mix); rows R:R+2 hold b1/b4
        y2e_pool = ctx.enter_context(tc.tile_pool(name="y2e", bufs=2))
        y2e_tiles = []
        for i in range(2):
            t = y2e_pool.tile([R + 2, SGS * COUT], bf16, tag=f"y2e{i}")
            nc.sync.dma_start(out=t[R:R + 2, :], in_=b1b4_d)
            y2e_tiles.append(t)
        y2et_pool = ctx.enter_context(tc.tile_pool(name="y2et", bufs=1))
        y2et = y2et_pool.tile([RT + 2, COUT], bf16)
        nc.sync.dma_start(out=y2et[RT:RT + 2, :], in_=b1b4_d[:, 0:COUT])

        xin_pool = ctx.enter_context(tc.tile_pool(name="xin", bufs=3))
        fout_pool = ctx.enter_context(tc.tile_pool(name="fout", bufs=3))
        h_pool = ctx.enter_context(tc.tile_pool(name="h", bufs=2))
        y1u_pool = ctx.enter_context(tc.tile_pool(name="y1u", bufs=2))
        lk_pool = ctx.enter_context(tc.tile_pool(name="lk", bufs=2))

        psH_pool = ctx.enter_context(tc.tile_pool(name="psH", bufs=2, space="PSUM"))
        psA_pool = ctx.enter_context(tc.tile_pool(name="psA", bufs=2, space="PSUM"))
        psB_pool = ctx.enter_context(tc.tile_pool(name="psB", bufs=2, space="PSUM"))
        psF_pool = ctx.enter_context(tc.tile_pool(name="psF", bufs=2, space="PSUM"))

        # tiles: (col_start, ncols, n_supergroups)
        tiles = [(t * COLS_FULL, COLS_FULL, TILE_SG) for t in range(NT_FULL)]
        tiles.append((NT_FULL * COLS_FULL, COLS_LAST, 1))
        # supergroup s -> (tile_idx, col offset within tile)
        sg_map = []
        for ti, (c0, ncols, nsg) in enumerate(tiles):
            for k in range(nsg):
                sg_map.append((ti, k * RSG))

        xin_tiles = [None] * len(tiles)
        fout_tiles = [None] * len(tiles)
        h_tiles = [None] * len(tiles)

        def open_tile(ti):
            c0, ncols, _ = tiles[ti]
            tag = "x" if ncols == COLS_FULL else "xL"
            xt = xin_pool.tile([CIN, ncols], bf16, tag=tag, name=f"xin_{tag}")
            nc.sync.dma_start(out=xt[:], in_=x_d[:, c0:c0 + ncols])
            xin_tiles[ti] = xt
            tag = "f" if ncols == COLS_FULL else "fL"
            fout_tiles[ti] = fout_pool.tile(
                [CIN, ncols], bf16, tag=tag, name=f"fout_{tag}")
            tag = "h" if ncols == COLS_FULL else "hL"
            h_tiles[ti] = h_pool.tile([MID, ncols], bf16, tag=tag, name=f"h_{tag}")

        def emit_w2(s):
            ti, off = sg_map[s]
            xt, ht = xin_tiles[ti], h_tiles[ti]
            psH = psH_pool.tile([MID, RSG], f32, tag="psH")
            nc.tensor.matmul(psH[:], lhsT=w2t_sb[:], rhs=xt[:, off:off + RSG],
                             start=True, stop=True)
            leaky(ht[:, off:off + RSG], psH)

        # stage 2 state: psF + supergroup id, lagged by one iteration
        pend = []

        def emit_front(s):
            """G1/G2 matmuls + copies for supergroup s."""
            ti, off = sg_map[s]
            xt, ht = xin_tiles[ti], h_tiles[ti]
            psA = psA_pool.tile([R, SGS * COUT], f32, tag="psA")
            for i in range(SGS):
                nc.tensor.matmul(
                    psA[:, i * COUT:(i + 1) * COUT],
                    lhsT=xt[:, off + i * R:off + (i + 1) * R], rhs=w1_sb[:],
                    start=True, stop=True)
            y1u = y1u_pool.tile([R, SGS * COUT], bf16, tag="y1u")
            nc.vector.tensor_copy(y1u[:], psA[:])
            psB = psB_pool.tile([R, SGS * COUT], f32, tag="psB")
            for i in range(SGS):
                nc.tensor.matmul(
                    psB[:, i * COUT:(i + 1) * COUT],
                    lhsT=ht[:, off + i * R:off + (i + 1) * R], rhs=w4t_sb[:],
                    start=True, stop=True)
            y2e = y2e_tiles[s % 2]
            nc.scalar.copy(y2e[0:R, :], psB[:])
            pend.append((s, y1u, y2e))

        def emit_back():
            """Mixing transposes + residual add for the oldest pending SG."""
            s, y1u, y2e = pend.pop(0)
            ti, off = sg_map[s]
            xt, ft = xin_tiles[ti], fout_tiles[ti]
            psF = psF_pool.tile([COUT, RSG], f32, tag="psF")
            for i in range(SGS):
                nc.tensor.matmul(
                    psF[:, i * R:(i + 1) * R],
                    lhsT=y1u[:, i * COUT:(i + 1) * COUT], rhs=bdM_sb[:],
                    start=True, stop=False, skip_group_check=True)
                nc.tensor.matmul(
                    psF[:, i * R:(i + 1) * R],
                    lhsT=y2e[:, i * COUT:(i + 1) * COUT], rhs=m2e_sb[:],
                    start=False, stop=True, skip_group_check=True)
            nc.vector.tensor_add(ft[:, off:off + RSG], psF[:], xt[:, off:off + RSG])

        def close_tile(ti):
            c0, ncols, _ = tiles[ti]
            nc.scalar.dma_start(out=o_d[:, c0:c0 + ncols], in_=fout_tiles[ti][:])

        open_tile(0)
        emit_w2(0)
        for s in range(NSG):
            ti = sg_map[s][0]
            if s + 1 < NSG:
                if sg_map[s + 1][0] != ti and xin_tiles[sg_map[s + 1][0]] is None:
                    open_tile(sg_map[s + 1][0])
                emit_w2(s + 1)
            emit_front(s)
            if pend and pend[0][0] < s:
                emit_back()
        while pend:
            emit_back()

        # ---- tail group: 4 batches / 68 rows, in the last tile ----
        ti = len(tiles) - 1
        xt, ht, ft = xin_tiles[ti], h_tiles[ti], fout_tiles[ti]
        off = RSG
        psHt = psH_pool.tile([MID, RSG], f32, tag="psH")
        psH = psHt[:, 0:RT]
        nc.tensor.matmul(psH, lhsT=w2t_sb[:], rhs=xt[:, off:off + RT],
                         start=True, stop=True)
        leaky(ht[:, off:off + RT], psH)
        psAt = psA_pool.tile([R, SGS * COUT], f32, tag="psA")
        psA = psAt[0:RT, 0:COUT]
        nc.tensor.matmul(psA, lhsT=xt[:, off:off + RT], rhs=w1_sb[:],
                         start=True, stop=True)
        y1u = y1u_pool.tile([RT, COUT], bf16, tag="y1ut")
        nc.vector.tensor_copy(y1u[:], psA)
        psBt = psB_pool.tile([R, SGS * COUT], f32, tag="psB")
        psB = psBt[0:RT, 0:COUT]
        nc.tensor.matmul(psB, lhsT=ht[:, off:off + RT], rhs=w4t_sb[:],
                         start=True, stop=True)
        nc.scalar.copy(y2et[0:RT, :], psB)
        psFt = psF_pool.tile([COUT, RSG], f32, tag="psF")
        psF = psFt[:, 0:RT]
        nc.tensor.matmul(psF, lhsT=y1u[:], rhs=bdM4_sb[:],
                         start=True, stop=False, skip_group_check=True)
        nc.tensor.matmul(psF, lhsT=y2et[:], rhs=m2e4_sb[:],
                         start=False, stop=True, skip_group_check=True)
        nc.vector.tensor_add(ft[:, off:off + RT], psF, xt[:, off:off + RT])

        for ti in range(len(tiles)):
            close_tile(ti)

    nc.compile()
    return nc


def _host_consts(inputs):
    bf = ml_dtypes.bfloat16
    M = _gcn_matrix(np.asarray(inputs["edge_index"]), np.asarray(inputs["edge_weight"]))
    adj = np.asarray(inputs["adj"], np.float32)
    bdM, m2e = _mix_consts(M, adj, G)
    bdM4, m2e4 = _mix_consts(M, adj, GT)
    W1 = np.asarray(inputs["W1"], np.float32)
    W2 = np.asarray(inputs["W2"], np.float32)
    W4 = np.asarray(inputs["W4"], np.float32)
    b1 = np.asarray(inputs["b1"], np.float32)
    b2 = np.asarray(inputs["b2"], np.float32)
    b4 = np.asarray(inputs["b4"], np.float32)
    b1b4 = np.stack([np.tile(b1, SGS), np.tile(b4, SGS)])
    return {
        "bdM": bdM.astype(bf),
        "m2e": m2e.astype(bf),
        "bdM4": bdM4.astype(bf),
        "m2e4": m2e4.astype(bf),
        "w1": np.ascontiguousarray(W1).astype(bf),
        "w2t": np.ascontiguousarray(W2.T).astype(bf),
        "w4t": np.ascontiguousarray(W4.T).astype(bf),
        "b2": np.ascontiguousarray(b2[:, None]),
        "ab2": np.ascontiguousarray(SLOPE * b2[:, None]),
        "b1b4": b1b4.astype(bf),
    }


def _shard_x(vector: np.ndarray) -> np.ndarray:
    """Full [B, J, CIN] fp32 -> [N_CORES, CIN, ROWS] bf16 channel-major."""
    bf = ml_dtypes.bfloat16
    v = np.asarray(vector, np.float32).reshape(N_CORES, ROWS, CIN)
    return v.transpose(0, 2, 1).astype(bf)


def _assemble_out(outs) -> np.ndarray:
    """list of [CIN, ROWS] bf16 -> [B, J, CIN] fp32."""
    stacked = np.stack(outs)  # [N_CORES, CIN, ROWS]
    return (
        stacked.transpose(0, 2, 1)
        .astype(np.float32)
        .reshape(B, J, CIN)
    )


def kernel(**inputs) -> np.ndarray:
    from concourse.bass_utils import run_bass_kernel_spmd

    if "nc" not in _CACHE:
        _CACHE["nc"] = _build_bass()
    nc = _CACHE["nc"]

    consts = _host_consts(inputs)
    xs = _shard_x(inputs["vector"])
    in_maps = []
    for c in range(N_CORES):
        m = dict(consts)
        m["x"] = xs[c]
        in_maps.append(m)

    res = run_bass_kernel_spmd(nc, in_maps, core_ids=list(range(N_CORES)))
    return _assemble_out([res.results[c]["out"] for c in range(N_CORES)])


# revision 11
# speedup vs baseline: 3.4106x; 1.1503x over previous
"""Bone_Direction_GCN fused kernel for 8 Trainium2 NeuronCores.

Data-parallel over the batch dim: each core processes 2048 of 16384 batches.
x is shipped to the device as bf16 channel-major [CIN, rows] so both the input
and output DMAs move large contiguous per-partition chunks (~7.6KB packets).
Graph mixing (GCN conv + dense-adj einsum) is expressed as block-diagonal
"mixing transpose" matmuls over groups of 7 batches (7*17 = 119 rows), which
also return the result to channel-major layout for the residual add and the
bf16 channel-major output store.
"""

import sys

sys.path.insert(0, "/opt/trn_rl_repo")

import numpy as np
import ml_dtypes

B, J, E = 16384, 17, 32
CIN, COUT = 128, 128
MID = COUT // 2
PROP = 0.5
SLOPE = 0.01

N_CORES = 8
BC = B // N_CORES          # batches per core (2048)
ROWS = BC * J              # rows per core (34816)
G = 7                      # batches per group
R = G * J                  # rows per group (119)
NG = BC // G               # full groups per core (292)
GT = BC - NG * G           # tail batches (4)
RT = GT * J                # tail rows (68)
SGS = 4                    # groups per supergroup
RSG = SGS * R              # cols per supergroup (476)
NSG = NG // SGS            # supergroups per core (73)
TILE_SG = 8                # supergroups per DMA tile
NT_FULL = (NSG - 1) // TILE_SG  # 9 full tiles; last tile = 1 SG + tail
COLS_FULL = TILE_SG * RSG  # 3808
COLS_LAST = RSG + RT       # 544

assert NT_FULL * COLS_FULL + COLS_LAST == ROWS

_CACHE = {}


def _gcn_matrix(edge_index: np.ndarray, edge_weight: np.ndarray) -> np.ndarray:
    """Dense normalized GCN operator M with out[i] = sum_j M[i, j] * x[j]."""
    row = edge_index[0].astype(np.int64)
    col = edge_index[1].astype(np.int64)
    loop = np.arange(J, dtype=np.int64)
    row_f = np.concatenate([row, loop])
    col_f = np.concatenate([col, loop])
    w_f = np.concatenate([edge_weight.astype(np.float32), np.ones(J, np.float32)])
    deg = np.zeros(J, np.float32)
    np.add.at(deg, col_f, w_f)
    safe = np.where(deg > 0, deg, 1.0).astype(np.float32)
    dis = np.where(deg > 0, 1.0 / np.sqrt(safe), 0.0).astype(np.float32)
    norm = dis[row_f] * w_f * dis[col_f]
    M = np.zeros((J, J), np.float32)
    np.add.at(M, (col_f, row_f), norm)
    return M


def _block_diag(block: np.ndarray, n: int) -> np.ndarray:
    j = block.shape[0]
    out = np.zeros((n * j, n * j), block.dtype)
    for g in range(n):
        out[g * j:(g + 1) * j, g * j:(g + 1) * j] = block
    return out


def _mix_consts(M: np.ndarray, adj: np.ndarray, g: int):
    """bdM [g*17, g*17] = blockdiag(M.T); m2e [g*17+2, g*17] = mix2 + bias rows.

    psF[c, r'] += sum_r y1u[r, c] * bdM[r, r']   (GCN mix, row r -> row r')
    psF[c, r'] += sum_k y2e[k, c] * m2e[k, r']   (adj mix + b1/b4 bias rows)
    """
    r = g * J
    bdM = _block_diag(M.T, g)
    mix2 = _block_diag(PROP * adj, g)
    ones_row = np.ones((1, r), np.float32)
    s_row = np.tile(PROP * adj.sum(axis=0), g)[None, :]
    m2e = np.concatenate([mix2, ones_row, s_row], axis=0)
    return bdM, m2e


def _build_bass(leaky_mode: str = "lrelu", **_ignored):
    import concourse.bacc as bacc
    import concourse.mybir as mybir
    import concourse.tile as tile
    from contextlib import ExitStack

    f32 = mybir.dt.float32
    bf16 = mybir.dt.bfloat16

    nc = bacc.Bacc("TRN2", target_bir_lowering=False, debug=False)

    x_d = nc.dram_tensor("x", [CIN, ROWS], bf16, kind="ExternalInput").ap()
    w1_d = nc.dram_tensor("w1", [CIN, COUT], bf16, kind="ExternalInput").ap()
    w2t_d = nc.dram_tensor("w2t", [CIN, MID], bf16, kind="ExternalInput").ap()
    w4t_d = nc.dram_tensor("w4t", [MID, COUT], bf16, kind="ExternalInput").ap()
    b2_d = nc.dram_tensor("b2", [MID, 1], f32, kind="ExternalInput").ap()
    ab2_d = nc.dram_tensor("ab2", [MID, 1], f32, kind="ExternalInput").ap()
    bdM_d = nc.dram_tensor("bdM", [R, R], bf16, kind="ExternalInput").ap()
    m2e_d = nc.dram_tensor("m2e", [R + 2, R], bf16, kind="ExternalInput").ap()
    bdM4_d = nc.dram_tensor("bdM4", [RT, RT], bf16, kind="ExternalInput").ap()
    m2e4_d = nc.dram_tensor("m2e4", [RT + 2, RT], bf16, kind="ExternalInput").ap()
    b1b4_d = nc.dram_tensor("b1b4", [2, 4 * COUT], bf16, kind="ExternalInput").ap()
    o_d = nc.dram_tensor("out", [CIN, ROWS], bf16, kind="ExternalOutput").ap()

    with ExitStack() as ctx:
        tc = ctx.enter_context(tile.TileContext(nc))

        const = ctx.enter_context(tc.tile_pool(name="const", bufs=1))
        w1_sb = const.tile_from(w1_d)
        w2t_sb = const.tile_from(w2t_d)
        w4t_sb = const.tile_from(w4t_d)
        b2_sb = const.tile_from(b2_d)
        ab2_sb = const.tile_from(ab2_d)
        bdM_sb = const.tile_from(bdM_d)
        m2e_sb = const.tile_from(m2e_d)
        bdM4_sb = const.tile_from(bdM4_d)
        m2e4_sb = const.tile_from(m2e4_d)

        def leaky(hbf, psH):
            if leaky_mode == "lrelu":
                nc.scalar.activation(
                    hbf[:], psH[:],
                    func=mybir.ActivationFunctionType.Lrelu,
                    bias=b2_sb[:], scale=1.0, alpha=SLOPE,
                )
            else:
                a = lk_pool.tile(list(psH.shape), bf16, tag="lk_a")
                nc.scalar.activation(
                    a[:], psH[:],
                    func=mybir.ActivationFunctionType.Identity,
                    bias=ab2_sb[:], scale=SLOPE,
                )
                nc.vector.scalar_tensor_tensor(
                    hbf[:], psH[:], b2_sb[:], a[:],
                    op0=mybir.AluOpType.add, op1=mybir.AluOpType.max,
                )

        # y2e tiles: rows 0:R hold y2 (d before bias/mix); rows R:R+2 hold b1/b4
        y2e_pool = ctx.enter_context(tc.tile_pool(name="y2e", bufs=2))
        y2e_tiles = []
        for i in range(2):
            t = y2e_pool.tile([R + 2, SGS * COUT], bf16, tag=f"y2e{i}")
            nc.sync.dma_start(out=t[R:R + 2, :], in_=b1b4_d)
            y2e_tiles.append(t)
        y2et_pool = ctx.enter_context(tc.tile_pool(name="y2et", bufs=1))
        y2et = y2et_pool.tile([RT + 2, COUT], bf16)
        nc.sync.dma_start(out=y2et[RT:RT + 2, :], in_=b1b4_d[:, 0:COUT])

        xin_pool = ctx.enter_context(tc.tile_pool(name="xin", bufs=3))
        fout_pool = ctx.enter_context(tc.tile_pool(name="fout", bufs=3))
        h_pool = ctx.enter_context(tc.tile_pool(name="h", bufs=3))
        y1u_pool = ctx.enter_context(tc.tile_pool(name="y1u", bufs=2))
        lk_pool = ctx.enter_context(tc.tile_pool(name="lk", bufs=2))

        psH_pool = ctx.enter_context(tc.tile_pool(name="psH", bufs=2, space="PSUM"))
        psA_pool = ctx.enter_context(tc.tile_pool(name="psA", bufs=2, space="PSUM"))
        psB_pool = ctx.enter_context(tc.tile_pool(name="psB", bufs=2, space="PSUM"))
        psF_pool = ctx.enter_context(tc.tile_pool(name="psF", bufs=2, space="PSUM"))

        # tiles: (col_start, ncols, n_supergroups)
        tiles = [(t * COLS_FULL, COLS_FULL, TILE_SG) for t in range(NT_FULL)]
        tiles.append((NT_FULL * COLS_FULL, COLS_LAST, 1))
        # supergroup s -> (tile_idx, col offset within tile)
        sg_map = []
        for ti, (c0, ncols, nsg) in enumerate(tiles):
            for k in range(nsg):
                sg_map.append((ti, k * RSG))

        xin_tiles = [None] * len(tiles)
        fout_tiles = [None] * len(tiles)
        h_tiles = [None] * len(tiles)

        def open_tile(ti):
            c0, ncols, _ = tiles[ti]
            tag = "x" if ncols == COLS_FULL else "xL"
            xt = xin_pool.tile([CIN, ncols], bf16, tag=tag, name=f"xin_{tag}")
            nc.sync.dma_start(out=xt[:], in_=x_d[:, c0:c0 + ncols])
            xin_tiles[ti] = xt
            tag = "f" if ncols == COLS_FULL else "fL"
            fout_tiles[ti] = fout_pool.tile(
                [CIN, ncols], bf16, tag=tag, name=f"fout_{tag}")
            tag = "h" if ncols == COLS_FULL else "hL"
            h_tiles[ti] = h_pool.tile([MID, ncols], bf16, tag=tag, name=f"h_{tag}")

        def emit_w2(s):
            ti, off = sg_map[s]
            xt, ht = xin_tiles[ti], h_tiles[ti]
            psH = psH_pool.tile([MID, RSG], f32, tag="psH")
            nc.tensor.matmul(psH[:], lhsT=w2t_sb[:], rhs=xt[:, off:off + RSG],
                             start=True, stop=True)
            leaky(ht[:, off:off + RSG], psH)

        # stage 2 state: psF + supergroup id, lagged by one iteration
        pend = []
        adds_left = [nsg for _, _, nsg in tiles]
        adds_left[-1] += 1  # tail group counts as one more add in the last tile

        def emit_front(s):
            """G1/G2 matmuls + copies for supergroup s."""
            ti, off = sg_map[s]
            xt, ht = xin_tiles[ti], h_tiles[ti]
            psA = psA_pool.tile([R, SGS * COUT], f32, tag="psA")
            for i in range(SGS):
                nc.tensor.matmul(
                    psA[:, i * COUT:(i + 1) * COUT],
                    lhsT=xt[:, off + i * R:off + (i + 1) * R], rhs=w1_sb[:],
                    start=True, stop=True)
            y1u = y1u_pool.tile([R, SGS * COUT], bf16, tag="y1u")
            nc.vector.tensor_copy(y1u[:], psA[:])
            psB = psB_pool.tile([R, SGS * COUT], f32, tag="psB")
            for i in range(SGS):
                nc.tensor.matmul(
                    psB[:, i * COUT:(i + 1) * COUT],
                    lhsT=ht[:, off + i * R:off + (i + 1) * R], rhs=w4t_sb[:],
                    start=True, stop=True)
            y2e = y2e_tiles[s % 2]
            nc.scalar.copy(y2e[0:R, :], psB[:])
            pend.append((s, y1u, y2e))

        def emit_back():
            """Mixing transposes + residual add for the oldest pending SG."""
            s, y1u, y2e = pend.pop(0)
            ti, off = sg_map[s]
            xt, ft = xin_tiles[ti], fout_tiles[ti]
            psF = psF_pool.tile([COUT, RSG], f32, tag="psF")
            for i in range(SGS):
                nc.tensor.matmul(
                    psF[:, i * R:(i + 1) * R],
                    lhsT=y1u[:, i * COUT:(i + 1) * COUT], rhs=bdM_sb[:],
                    start=True, stop=False, skip_group_check=True)
                nc.tensor.matmul(
                    psF[:, i * R:(i + 1) * R],
                    lhsT=y2e[:, i * COUT:(i + 1) * COUT], rhs=m2e_sb[:],
                    start=False, stop=True, skip_group_check=True)
            nc.vector.tensor_add(ft[:, off:off + RSG], psF[:], xt[:, off:off + RSG])
            adds_left[ti] -= 1
            if adds_left[ti] == 0:
                close_tile(ti)

        def close_tile(ti):
            c0, ncols, _ = tiles[ti]
            nc.scalar.dma_start(out=o_d[:, c0:c0 + ncols], in_=fout_tiles[ti][:])

        open_tile(0)
        emit_w2(0)
        if NSG > 1:
            if sg_map[1][0] != 0 and xin_tiles[sg_map[1][0]] is None:
                open_tile(sg_map[1][0])
            emit_w2(1)
        for s in range(NSG):
            emit_front(s)
            if pend and pend[0][0] < s:
                emit_back()
            if s + 2 < NSG:
                if xin_tiles[sg_map[s + 2][0]] is None:
                    open_tile(sg_map[s + 2][0])
                emit_w2(s + 2)
        while pend:
            emit_back()

        # ---- tail group: 4 batches / 68 rows, in the last tile ----
        ti = len(tiles) - 1
        xt, ht, ft = xin_tiles[ti], h_tiles[ti], fout_tiles[ti]
        off = RSG
        psHt = psH_pool.tile([MID, RSG], f32, tag="psH")
        psH = psHt[:, 0:RT]
        nc.tensor.matmul(psH, lhsT=w2t_sb[:], rhs=xt[:, off:off + RT],
                         start=True, stop=True)
        leaky(ht[:, off:off + RT], psH)
        psAt = psA_pool.tile([R, SGS * COUT], f32, tag="psA")
        psA = psAt[0:RT, 0:COUT]
        nc.tensor.matmul(psA, lhsT=xt[:, off:off + RT], rhs=w1_sb[:],
                         start=True, stop=True)
        y1u = y1u_pool.tile([RT, COUT], bf16, tag="y1ut")
        nc.vector.tensor_copy(y1u[:], psA)
        psBt = psB_pool.tile([R, SGS * COUT], f32, tag="psB")
        psB = psBt[0:RT, 0:COUT]
        nc.tensor.matmul(psB, lhsT=ht[:, off:off + RT], rhs=w4t_sb[:],
                         start=True, stop=True)
        nc.scalar.copy(y2et[0:RT, :], psB)
        psFt = psF_pool.tile([COUT, RSG], f32, tag="psF")
        psF = psFt[:, 0:RT]
        nc.tensor.matmul(psF, lhsT=y1u[:], rhs=bdM4_sb[:],
                         start=True, stop=False, skip_group_check=True)
        nc.tensor.matmul(psF, lhsT=y2et[:], rhs=m2e4_sb[:],
                         start=False, stop=True, skip_group_check=True)
        nc.vector.tensor_add(ft[:, off:off + RT], psF, xt[:, off:off + RT])
        adds_left[ti] -= 1
        assert adds_left[ti] == 0 and not any(adds_left)
        close_tile(ti)

    nc.compile()
    return nc


def _host_consts(inputs):
    bf = ml_dtypes.bfloat16
    M = _gcn_matrix(np.asarray(inputs["edge_index"]), np.asarray(inputs["edge_weight"]))
    adj = np.asarray(inputs["adj"], np.float32)
    bdM, m2e = _mix_consts(M, adj, G)
    bdM4, m2e4 = _mix_consts(M, adj, GT)
    W1 = np.asarray(inputs["W1"], np.float32)
    W2 = np.asarray(inputs["W2"], np.float32)
    W4 = np.asarray(inputs["W4"], np.float32)
    b1 = np.asarray(inputs["b1"], np.float32)
    b2 = np.asarray(inputs["b2"], np.float32)
    b4 = np.asarray(inputs["b4"], np.float32)
    b1b4 = np.stack([np.tile(b1, SGS), np.tile(b4, SGS)])
    return {
        "bdM": bdM.astype(bf),
        "m2e": m2e.astype(bf),
        "bdM4": bdM4.astype(bf),
        "m2e4": m2e4.astype(bf),
        "w1": np.ascontiguousarray(W1).astype(bf),
        "w2t": np.ascontiguousarray(W2.T).astype(bf),
        "w4t": np.ascontiguousarray(W4.T).astype(bf),
        "b2": np.ascontiguousarray(b2[:, None]),
        "ab2": np.ascontiguousarray(SLOPE * b2[:, None]),
        "b1b4": b1b4.astype(bf),
    }


def _shard_x(vector: np.ndarray) -> np.ndarray:
    """Full [B, J, CIN] fp32 -> [N_CORES, CIN, ROWS] bf16 channel-major."""
    bf = ml_dtypes.bfloat16
    v = np.asarray(vector, np.float32).reshape(N_CORES, ROWS, CIN)
    return v.transpose(0, 2, 1).astype(bf)


def _assemble_out(outs) -> np.ndarray:
    """list of [CIN, ROWS] bf16 -> [B, J, CIN] fp32."""
    stacked = np.stack(outs)  # [N_CORES, CIN, ROWS]
    return (
        stacked.transpose(0, 2, 1)
        .astype(np.float32)
        .reshape(B, J, CIN)
    )


def kernel(**inputs) -> np.ndarray:
    from concourse.bass_utils import run_bass_kernel_spmd

    if "nc" not in _CACHE:
        _CACHE["nc"] = _build_bass()
    nc = _CACHE["nc"]

    consts = _host_consts(inputs)
    xs = _shard_x(inputs["vector"])
    in_maps = []
    for c in range(N_CORES):
        m = dict(consts)
        m["x"] = xs[c]
        in_maps.append(m)

    res = run_bass_kernel_spmd(nc, in_maps, core_ids=list(range(N_CORES)))
    return _assemble_out([res.results[c]["out"] for c in range(N_CORES)])
